# revision 21
# baseline (speedup 1.0000x reference)
"""Trainium2 Bass kernel for the dual-modality dense transformer block.

Problem (hardcoded shapes): B=8, L=1024, H=512, NH=8, HD=64.
  - 6 linear projections (q/k/v for img and txt streams)
  - 4 full attentions: (q_img,KV_img), (q_txt,KV_txt), (q_img,KV_txt), (q_txt,KV_img)
  - out_img/out_txt linears on the averaged contexts, concat + cat linear
  - attention pooling (nn.MultiheadAttention-style) + out_proj

Sharding: pure data-parallel over batch B=8 across the 8 NeuronCores.

Key device-level design (v2):
  - q/k/v and in_proj-q/k projections run in fp8e4 with
    perf_mode=DoubleRowSwInterleave (stationary host-interleaved, 2 k-planes
    per pass -> ~4x fewer PE cycles on those units). x/t inputs are shipped
    pre-quantized fp8 (x16) in both plane-major (moving) and
    interleaved-reversed (stationary) layouts. Weights fp8 (x256); evictions
    descale by 1/4096 and add the bias.
  - q/k tiles are stored fp8 (natural scale); QK matmuls are plain fp8
    (bf16-rate) with two heads packed per PE pass via tile_position row
    groups, which run concurrently (small-K row tiling).
  - exp(score) is split across two engines per (ih,p) block: ACT runs the
    exact table exp (scale=1/8 folds the score descale), DVE runs a custom
    cubic-polynomial (p(u/32)^4) single-instruction approximation. Each
    block's query rows use one implementation so the constant factor
    cancels in softmax.
  - softmax denominators come free from the PV matmul via a ones-column in
    the V tile (M=65); reciprocal on DVE (fast bit-trick op), broadcast to
    64 partitions by the (otherwise idle) GPSIMD partition_broadcast.
  - V-projection biases are folded into the *downstream* projection biases
    on the host (b_oi += 0.5*(b_v_img+b_v_txt) @ W_oi.T), so V tiles carry
    no bias matmuls.
  - bf16 everywhere else, fp32 PSUM. Measured accuracy vs the fp32
    reference: ~5e-3 of output absmax (budget 2e-2).
"""

import numpy as np
import ml_dtypes

import concourse.bass as bass
import concourse.tile as tile
from concourse import bacc, mybir
from concourse.bass_utils import run_bass_kernel_spmd
from concourse.dve_ops import RECIP_APPROX_FAST_CONSTS, RECIPROCAL_APPROX_FAST

B, L, H, NH, HD = 8, 1024, 512, 8, 64
BF = mybir.dt.bfloat16
F32 = mybir.dt.float32
F8 = mybir.dt.float8e4
Exp = mybir.ActivationFunctionType.Exp
Ident = mybir.ActivationFunctionType.Identity
bf16 = ml_dtypes.bfloat16
f8 = ml_dtypes.float8_e4m3
DRI = mybir.MatmulPerfMode.DoubleRowSwInterleave

N_CORES = 8

# ---------------- custom DVE exp op (registered at import) ----------------
from concourse.dve_spec import Spec, Src0, C0, C1, C2, One, lower as _dve_lower, _has_src1
from concourse.dve_ops import DveOp, OPS as _DVE_OPS, CUSTOM_DVE_SPECS as _DVE_SPECS
from concourse.dve_ops import _SUB_OPCODE_FOR_NAME, _CUSTOM_DVE_ROW_BASE
from concourse.dve_uop import DveOpSpec


def _make_exp_op():
    if "EXP4_POLY_ANT" in _SUB_OPCODE_FOR_NAME:
        return next(o for o in _DVE_OPS if o.name == "EXP4_POLY_ANT")
    u = Src0
    p = ((C2 * u + C1) * u + C0) * u + One
    body = (p * p) * (p * p)

    def ref(in0, in1, s0, s1, imm2):
        x = in0.astype(np.float32)
        q = ((imm2 * x + s1) * x + s0) * x + 1.0
        q2 = q * q
        return q2 * q2

    spec = Spec(body=body, reference=ref)
    name = "EXP4_POLY_ANT"
    opcode = _CUSTOM_DVE_ROW_BASE + len(_DVE_OPS)
    shas = {}
    for ver in ("v3",):
        uops = _dve_lower(spec, ver=ver)
        shas[ver] = DveOpSpec(
            name=name, opcode=opcode, uops=uops, rd1_en=_has_src1(spec)
        ).sha(ver)
    op = DveOp(name, spec, subdim=False, uops_sha=shas)
    _DVE_OPS.append(op)
    _DVE_SPECS[name] = spec
    _SUB_OPCODE_FOR_NAME[name] = opcode
    return op


EXP4 = _make_exp_op()

# cubic fit of exp(x) ~= p(x/4)^4 on |x|<=3.8 (density-weighted toward the
# observed score distribution); c0 normalized to 1 (One) - the residual
# constant factor cancels in softmax row-normalization.
_EC = np.array([0.99919218, 1.00539871, 0.52221469, 0.15490101])
_EC = _EC / _EC[0]
# psum scores arrive as u = 8*score (q,k at natural scale, no 1/sqrt(HD)
# fold); y = score/4 = u/32
_G = 1.0 / 32.0
EXP_S0, EXP_S1, EXP_IMM2 = float(_EC[1] * _G), float(_EC[2] * _G ** 2), float(_EC[3] * _G ** 3)
ACT_EXP_SCALE = 1.0 / 8.0


def _dve_exp_block(attn_idx, ih, p):
    """Which (ih, p) exp blocks run on DVE (the rest on ACT)."""
    return (2 * ih + p + attn_idx) % 4 == 3


def _emit(tc, d):
    nc = tc.nc
    import contextlib

    ctx = contextlib.ExitStack()
    with ctx:
        const = ctx.enter_context(tc.tile_pool(name="const", bufs=1))
        acts = ctx.enter_context(tc.tile_pool(name="acts", bufs=1))
        spool = ctx.enter_context(tc.tile_pool(name="spool", bufs=2))
        opool = ctx.enter_context(tc.tile_pool(name="opool", bufs=1))
        expool = ctx.enter_context(tc.tile_pool(name="expool", bufs=2))
        small = ctx.enter_context(tc.tile_pool(name="small", bufs=2))
        pmm = ctx.enter_context(tc.tile_pool(name="pmm", bufs=2, space="PSUM"))
        pctx = ctx.enter_context(tc.tile_pool(name="pctx", bufs=2, space="PSUM"))

        def load(name, shape, dt, pool=const, tag=None, split=None):
            t = pool.tile(shape, dt, tag=tag or name)
            if split is None:
                nc.sync.dma_start(out=t, in_=d[name])
            else:
                # split the transfer across DMA queues on dim 1
                for c in range(shape[1]):
                    nc.sync.dma_start(out=t[:, c], in_=d[name][:, c])
            return t

        # ---- loads in first-use order ----
        x8 = load("x8", [128, 2, 2, L], F8, pool=acts, split=True)
        w8i_qim = load("w8i_qim", [128, 2, 4, 256], F8)
        b_qim = load("b_qim", [128, 4], F32)
        w8i_kim = load("w8i_kim", [128, 2, 4, 256], F8)
        b_kim = load("b_kim", [128, 4], F32)
        x8i = load("x8i", [128, 2, 8, 256], F8, pool=acts, split=True)
        w8v_im = load("w8v_im", [128, 2, 2, 512], F8)
        t8 = load("t8", [128, 2, 2, L], F8, pool=acts, split=True)
        w8i_qtx = load("w8i_qtx", [128, 2, 4, 256], F8)
        b_qtx = load("b_qtx", [128, 4], F32)
        w8i_ktx = load("w8i_ktx", [128, 2, 4, 256], F8)
        b_ktx = load("b_ktx", [128, 4], F32)
        t8i = load("t8i", [128, 2, 8, 256], F8, pool=acts, split=True)
        w8v_tx = load("w8v_tx", [128, 2, 2, 512], F8)
        w_oim = load("w_oim", [128, 4, 512], BF, split=True)
        b_oim = load("b_oim", [128, 4], F32)
        w_otx = load("w_otx", [128, 4, 512], BF, split=True)
        b_otx = load("b_otx", [128, 4], F32)
        w_cat = load("w_cat", [128, 8, 512], BF, split=True)
        b_cat = load("b_cat", [128, 4], F32)
        w8i_ipq = load("w8i_ipq", [128, 2, 4, 256], F8)
        w8i_ipk = load("w8i_ipk", [128, 2, 4, 256], F8)
        b_ipqk = load("b_ipqk", [128, 8], F32)
        w_ipv = load("w_ipv", [128, 4, 512], BF, split=True)
        w_op = load("w_op", [128, 4, 512], BF, split=True)
        r_op = load("r_op", [1, 512], BF)

        ones_row = const.tile([1, 128], BF, tag="ones_row")
        nc.vector.memset(ones_row, 1.0)

        # ---- helpers ----
        def evict(eng, out, ps, scale, biascol):
            if eng == "act":
                nc.scalar.activation(out, ps, Ident, bias=biascol, scale=scale)
            else:
                if biascol is None:
                    if scale == 1.0:
                        nc.vector.tensor_copy(out=out, in_=ps)
                    else:
                        nc.vector.tensor_scalar_mul(out, ps, scale)
                else:
                    nc.vector.tensor_scalar(
                        out=out, in0=ps, scalar1=scale, scalar2=biascol,
                        op0=mybir.AluOpType.mult, op1=mybir.AluOpType.add,
                    )

        def proj_T8_m(dst, x8t, w8i, bias, bias_off, m, eng="act"):
            ps = pmm.tile([128, 1024], F32, tag="mm")
            for n in range(2):
                for kc in range(2):
                    nc.tensor.matmul(
                        ps[:, n * 512 : (n + 1) * 512],
                        w8i[:, kc, m, :].rearrange("p (m2 t) -> p m2 t", t=2),
                        x8t[:, kc, :, n * 512 : (n + 1) * 512],
                        start=(kc == 0),
                        stop=(kc == 1),
                        perf_mode=DRI,
                    )
            evict(eng, dst[:, m, :], ps, 1.0 / 4096.0,
                  bias[:, bias_off + m : bias_off + m + 1] if bias is not None else None)

        def proj_T8(dst, x8t, w8i, bias, bias_off, eng="act"):
            """fp8 DRI feature-major linear: dst[:, m, :] ~ fp8/bf16 [128,4,L]."""
            for m in range(4):
                proj_T8_m(dst, x8t, w8i, bias, bias_off, m, eng)

        def proj_N8_lc2(dst, x8it, w8v, lc2):
            ps = pmm.tile([128, 1024], F32, tag="mm")
            for h in range(2):
                lc = lc2 * 2 + h
                for kc in range(2):
                    nc.tensor.matmul(
                        ps[:, h * 512 : (h + 1) * 512],
                        x8it[:, kc, lc, :].rearrange("p (m2 t) -> p m2 t", t=2),
                        w8v[:, kc, :, :],
                        start=(kc == 0),
                        stop=(kc == 1),
                        perf_mode=DRI,
                        skip_group_check=True,
                    )
            nc.vector.tensor_scalar_mul(
                dst[:, lc2 * 2 : lc2 * 2 + 2, :, 0:64],
                ps.rearrange("p (a b) -> p a b", a=2),
                1.0 / 4096.0,
            )

        def proj_N8(dst, x8it, w8v):
            """fp8 DRI natural-orientation v-projection into ones-augmented
            layout dst [128, 8(jt), 8(lc-ish), 65]; no bias (host-folded)."""
            for lc2 in range(4):
                proj_N8_lc2(dst, x8it, w8v, lc2)

        def proj_T_m(dst, src, nk, w, bias, bias_off, m, eng="dve"):
            ps = pmm.tile([128, 1024], F32, tag="mm")
            for n in range(2):
                for k in range(nk):
                    nc.tensor.matmul(
                        ps[:, n * 512 : (n + 1) * 512],
                        w[:, k, m * 128 : (m + 1) * 128],
                        src[:, k, n * 512 : (n + 1) * 512],
                        start=(k == 0),
                        stop=(k == nk - 1),
                    )
            evict(eng, dst[:, m, :], ps, 1.0, bias[:, bias_off + m : bias_off + m + 1])

        def proj_T(dst, src, nk, w, bias, bias_off, eng="dve"):
            """bf16 feature-major linear (as baseline)."""
            for m in range(4):
                proj_T_m(dst, src, nk, w, bias, bias_off, m, eng)

        def proj_N(dst, src, w):
            """bf16 natural-orientation projection (pooling v), no bias."""
            for lc2 in range(4):
                ps = pmm.tile([128, 1024], F32, tag="mm")
                for h in range(2):
                    lc = lc2 * 2 + h
                    for k in range(4):
                        nc.tensor.matmul(
                            ps[:, h * 512 : (h + 1) * 512],
                            src[:, k, lc * 128 : (lc + 1) * 128],
                            w[:, k, :],
                            start=(k == 0),
                            stop=(k == 3),
                            skip_group_check=True,
                        )
                nc.vector.tensor_copy(
                    out=dst[:, lc2 * 2 : lc2 * 2 + 2, :, 0:64],
                    in_=ps.rearrange("p (a b) -> p a b", a=2),
                )

        # deferred-normalize queue (depth 2: two ctx psum tiles in flight)
        pending = []
        exp_ctr = [0]

        def flush_one():
            if pending:
                pending.pop(0)()

        def flush_all():
            while pending:
                pending.pop(0)()

        def emit_block(st, ih, p):
            """One (ih, p) block of an attention: QK + exp + PV + queue the
            normalize. st = (qT, kT, vN, s_dst, first, scale)."""
            qT, kT, vN, s_dst, first, scale = st
            i0 = ih * 512
            use_dve = exp_ctr[0] % 4 == 3
            exp_ctr[0] += 1
            ex = expool.tile([128, 8, 1024], BF, tag="exp")
            for jt in range(8):
                ps = pmm.tile([128, 1024], F32, tag="mm")
                for hh in range(2):
                    nc.tensor.matmul(
                        ps[:, hh * 512 : (hh + 1) * 512],
                        kT[hh * 64 : (hh + 1) * 64, p, jt * 128 : (jt + 1) * 128],
                        qT[hh * 64 : (hh + 1) * 64, p, i0 : i0 + 512],
                        start=True,
                        stop=True,
                        tile_position=(hh * 64, 0),
                    )
                if use_dve:
                    nc.vector._custom_dve(
                        EXP4, out=ex[:, jt, :], in0=ps,
                        s0=EXP_S0, s1=EXP_S1, imm2=EXP_IMM2,
                    )
                else:
                    nc.scalar.activation(ex[:, jt, :], ps, Exp, scale=ACT_EXP_SCALE)
            if len(pending) >= 2:
                pending.pop(0)()
            cps = pctx.tile([128, 1024], F32, tag="ctx")
            for jt in range(8):
                for hh in range(2):
                    nc.tensor.matmul(
                        cps[0:65, hh * 512 : (hh + 1) * 512],
                        vN[:, jt, p * 2 + hh, :],
                        ex[:, jt, hh * 512 : (hh + 1) * 512],
                        start=(jt == 0),
                        stop=(jt == 7),
                    )

            def normalize(cps=cps, p=p, i0=i0, first=first, scale=scale, s_dst=s_dst):
                # scaled copy of the denominator rows to SBUF (the recip
                # bit-trick cannot read PSUM); scale=2 folds the reference's
                # 0.5 ctx averaging
                den = small.tile([1, 1024], F32, tag="den")
                nc.vector.tensor_scalar_mul(den, cps[64:65, :], scale)
                rc = small.tile([1, 1024], BF, tag="rc")
                cdve = RECIP_APPROX_FAST_CONSTS
                nc.vector._custom_dve(
                    RECIPROCAL_APPROX_FAST, out=rc, in0=den,
                    s0=cdve["s0"], s1=cdve["s1"], imm2=cdve["imm2"],
                )
                # partition-broadcast of the recips on GPSIMD (out tiles must
                # sit at partition base 0 - base-64 writes are broken)
                bcs0 = small.tile([64, 512], BF, tag="bcs0")
                bcs1 = small.tile([64, 512], BF, tag="bcs1")
                nc.gpsimd.partition_broadcast(bcs0, rc[0:1, 0:512])
                nc.gpsimd.partition_broadcast(bcs1, rc[0:1, 512:1024])
                o = s_dst[:, p, i0 : i0 + 512]
                if first:
                    nc.vector.tensor_mul(o[0:64, :], cps[0:64, 0:512], bcs0)
                    nc.vector.tensor_mul(o[64:128, :], cps[0:64, 512:1024], bcs1)
                else:
                    tmp = small.tile([128, 512], BF, tag="tmp")
                    nc.vector.tensor_mul(tmp[0:64, :], cps[0:64, 0:512], bcs0)
                    nc.vector.tensor_mul(tmp[64:128, :], cps[0:64, 512:1024], bcs1)
                    nc.vector.tensor_add(o, o, tmp)

            pending.append(normalize)

        def attention(st, fillers=None, start_slot=0):
            """Solo attention: 8 blocks with optional PE-filler closures
            emitted between blocks (leftovers drained at the end)."""
            fillers = list(fillers or [])
            for s in range(8):
                ih, p = s // 4, s % 4
                emit_block(st, ih, p)
                if fillers and s >= start_slot:
                    fillers.pop(0)()
            for f in fillers:
                f()

        def attention_pair(st_a, st_b, fillers=None):
            """Two independent attentions interleaved block-by-block; their
            exp streams keep both ACT and DVE busy while PE stays dense."""
            fillers = list(fillers or [])
            for ih in range(2):
                for p in range(4):
                    emit_block(st_a, ih, p)
                    emit_block(st_b, ih, p)
                    if fillers:
                        fillers.pop(0)()
            for f in fillers:
                f()

        # ---- the network ----
        q_im = acts.tile([128, 4, L], F8, tag="q_im")
        k_im = acts.tile([128, 4, L], F8, tag="k_im")
        v_im = acts.tile([128, 8, 8, 65], BF, tag="v_im")
        nc.vector.memset(v_im[:, :, :, 64:65], 1.0)
        q_tx = acts.tile([128, 4, L], F8, tag="q_tx")
        k_tx = acts.tile([128, 4, L], F8, tag="k_tx")
        v_tx = acts.tile([128, 8, 8, 65], BF, tag="v_tx")
        nc.vector.memset(v_tx[:, :, :, 64:65], 1.0)

        proj_T8(q_im, x8, w8i_qim, b_qim, 0, eng="act")
        proj_T8(k_im, x8, w8i_kim, b_kim, 0, eng="act")
        proj_N8(v_im, x8i, w8v_im)

        s_img = spool.tile([128, 4, L], BF, tag="s")
        s_txt = spool.tile([128, 4, L], BF, tag="s")

        # q_tx upfront so pair(A1, A4) can start; k_tx/v_tx are fillers
        proj_T8(q_tx, t8, w8i_qtx, b_qtx, 0, eng="act")

        def fT8(dst, x8t, w8i, bias, boff, ms, eng="act"):
            def f():
                for m in ms:
                    proj_T8_m(dst, x8t, w8i, bias, boff, m, eng)
            return f

        def fN8(dst, x8it, w8v, lc2s):
            def f():
                for lc2 in lc2s:
                    proj_N8_lc2(dst, x8it, w8v, lc2)
            return f

        p1_fillers = (
            [fT8(k_tx, t8, w8i_ktx, b_ktx, 0, [m]) for m in range(4)]
            + [fN8(v_tx, t8i, w8v_tx, [lc2]) for lc2 in range(4)]
        )
        st1 = (q_im, k_im, v_im, s_img, True, 2.0)   # ctx_img
        st4 = (q_tx, k_im, v_im, s_txt, True, 2.0)   # ctx_ti (first into s_txt)
        attention_pair(st1, st4, fillers=p1_fillers)

        st2 = (q_im, k_tx, v_tx, s_img, False, 2.0)  # ctx_it
        st3 = (q_tx, k_tx, v_tx, s_txt, False, 2.0)  # ctx_txt
        attention_pair(st2, st3)
        flush_all()

        cat_a = acts.tile([128, 4, L], BF, tag="cat_a")
        cat_b = acts.tile([128, 4, L], BF, tag="cat_b")
        proj_T(cat_a, s_img, 4, w_oim, b_oim, 0, eng="dve")
        proj_T(cat_b, s_txt, 4, w_otx, b_otx, 0, eng="act")

        # cat projection: dual eviction (bf16 out_t + fp8 out8 x16)
        out_t = opool.tile([128, 4, L], BF, tag="out")
        out8 = opool.tile([128, 2, 2, L], F8, tag="out8")
        for m in range(4):
            ps = pmm.tile([128, 1024], F32, tag="mm")
            for n in range(2):
                for k in range(8):
                    srck = cat_a if k < 4 else cat_b
                    nc.tensor.matmul(
                        ps[:, n * 512 : (n + 1) * 512],
                        w_cat[:, k, m * 128 : (m + 1) * 128],
                        srck[:, k % 4, n * 512 : (n + 1) * 512],
                        start=(k == 0),
                        stop=(k == 7),
                    )
            nc.vector.tensor_scalar_add(out_t[:, m, :], ps, b_cat[:, m : m + 1])
            # fp8 copy: (ps + b) * 16
            nc.vector.tensor_scalar(
                out=out8[:, m // 2, m % 2, :], in0=ps,
                scalar1=b_cat[:, m : m + 1], scalar2=16.0,
                op0=mybir.AluOpType.add, op1=mybir.AluOpType.mult,
            )

        q_pl = acts.tile([128, 4, L], F8, tag="q_im")
        k_pl = acts.tile([128, 4, L], F8, tag="q_tx")
        v_pl = acts.tile([128, 8, 8, 65], BF, tag="v_im")
        nc.vector.memset(v_pl[:, :, :, 64:65], 1.0)
        proj_T8(q_pl, out8, w8i_ipq, b_ipqk, 0, eng="act")
        proj_T8(k_pl, out8, w8i_ipk, b_ipqk, 4, eng="act")
        proj_N(v_pl, out_t, w_ipv)

        ctx_p = spool.tile([128, 4, L], BF, tag="s")

        def emit_out_proj(lcs):
            for lc in lcs:
                ps = pmm.tile([128, 1024], F32, tag="mm")
                for k in range(4):
                    nc.tensor.matmul(
                        ps[:, 0:512],
                        ctx_p[:, k, lc * 128 : (lc + 1) * 128],
                        w_op[:, k, :],
                        start=(k == 0),
                        stop=False,
                        skip_group_check=True,
                    )
                nc.tensor.matmul(
                    ps[:, 0:512], ones_row, r_op, start=False, stop=True,
                    skip_group_check=True,
                )
                res = small.tile([128, 512], F32, tag="res")
                nc.vector.tensor_copy(out=res, in_=ps[:, 0:512])
                nc.sync.dma_start(out=d["out"][lc * 128 : (lc + 1) * 128, :], in_=res)

        # pool attention: out_proj units become available per ih-half; with
        # the depth-2 normalize queue, ih0 is fully flushed after emitting
        # block (1,1) - attach lc 0-3 to the last slots, drain 4-7 after.
        st5 = (q_pl, k_pl, v_pl, ctx_p, True, 1.0)
        pool_fillers = [
            lambda: emit_out_proj([0, 1]),
            lambda: emit_out_proj([2, 3]),
        ]
        attention(st5, fillers=pool_fillers, start_slot=6)
        flush_all()
        emit_out_proj(range(4, 8))

        if "dbg_q_im" in d:
            for nm, t in (("dbg_q_im", q_im), ("dbg_k_im", k_im),
                          ("dbg_q_tx", q_tx), ("dbg_k_tx", k_tx)):
                nc.sync.dma_start(out=d[nm], in_=t)
            for nm, t in (("dbg_v_im", v_im), ("dbg_v_tx", v_tx)):
                nc.sync.dma_start(out=d[nm], in_=t)
            nc.sync.dma_start(out=d["dbg_s_img"], in_=s_img)
            nc.sync.dma_start(out=d["dbg_s_txt"], in_=s_txt)
            nc.sync.dma_start(out=d["dbg_out_t"], in_=out_t)
            nc.sync.dma_start(out=d["dbg_ctx_p"], in_=ctx_p)


_PROGRAM = None
DEBUG_DUMPS = False


def _build_program():
    global _PROGRAM
    if _PROGRAM is not None:
        return _PROGRAM
    nc = bacc.Bacc("TRN2", target_bir_lowering=False, debug=False)
    d = {}

    def din(name, shape, dt):
        d[name] = nc.dram_tensor(name, list(shape), dt, kind="ExternalInput").ap()

    din("x8", (128, 2, 2, L), F8)
    din("t8", (128, 2, 2, L), F8)
    din("x8i", (128, 2, 8, 256), F8)
    din("t8i", (128, 2, 8, 256), F8)
    for n in ("w8i_qim", "w8i_kim", "w8i_qtx", "w8i_ktx", "w8i_ipq", "w8i_ipk"):
        din(n, (128, 2, 4, 256), F8)
    for n in ("w8v_im", "w8v_tx"):
        din(n, (128, 2, 2, 512), F8)
    for n in ("w_oim", "w_otx", "w_ipv", "w_op"):
        din(n, (128, 4, 512), BF)
    din("w_cat", (128, 8, 512), BF)
    for n in ("b_qim", "b_kim", "b_qtx", "b_ktx", "b_oim", "b_otx", "b_cat"):
        din(n, (128, 4), F32)
    din("b_ipqk", (128, 8), F32)
    din("r_op", (1, 512), BF)
    d["out"] = nc.dram_tensor("out", [L, H], F32, kind="ExternalOutput").ap()
    if DEBUG_DUMPS:
        def dout(name, shape, dt):
            d[name] = nc.dram_tensor(name, list(shape), dt, kind="ExternalOutput").ap()
        for nm in ("dbg_q_im", "dbg_k_im", "dbg_q_tx", "dbg_k_tx"):
            dout(nm, (128, 4, L), F8)
        for nm in ("dbg_v_im", "dbg_v_tx"):
            dout(nm, (128, 8, 8, 65), BF)
        for nm in ("dbg_s_img", "dbg_s_txt", "dbg_out_t", "dbg_ctx_p"):
            dout(nm, (128, 4, L), BF)

    with tile.TileContext(nc) as tc:
        _emit(tc, d)
    nc.compile()
    _PROGRAM = nc
    return nc


def _interleave_stationary(a):
    """[128, pl2, nblk, 128] fp8-valued float -> [128, nblk, 256] interleaved
    (pair (plane0 col j, plane1 col j) adjacent, columns reversed)."""
    rev = a[:, :, :, ::-1]
    return rev.transpose(0, 2, 3, 1).reshape(a.shape[0], a.shape[2], 256)


def _prep_w8i(w):
    """w [H_out=512, H_in=512] -> DRI stationary [128, kc2, mb4, 256] fp8."""
    wt = np.ascontiguousarray(w.T) * 256.0  # [in, out]
    q = wt.astype(f8).astype(np.float32)
    r = q.reshape(2, 2, 128, 512).transpose(2, 0, 1, 3)  # [128, kc, pl, out]
    out = np.zeros((128, 2, 4, 256), np.float32)
    for kc in range(2):
        blk = r[:, kc].reshape(128, 2, 4, 128)  # [128, pl, mb, 128]
        out[:, kc] = _interleave_stationary(blk)
    return out.astype(f8)


def _prep_w8v(w):
    """w [H_out=512, H_in=512] -> DRI moving [128, kc2, pl2, 512] fp8."""
    wt = np.ascontiguousarray(w.T) * 256.0
    q = wt.astype(f8)
    return np.ascontiguousarray(
        q.reshape(2, 2, 128, 512).transpose(2, 0, 1, 3)
    )


def _prep_x8(x):
    """x [L, H] -> plane-major moving [128, kc2, pl2, L] fp8 and
    interleaved stationary [128, kc2, 8, 256] fp8 (both x16)."""
    xt = np.ascontiguousarray(x.T) * 16.0  # [H, L]
    q = xt.astype(f8)
    mov = np.ascontiguousarray(q.reshape(2, 2, 128, L).transpose(2, 0, 1, 3))
    qf = q.astype(np.float32)
    sta = np.zeros((128, 2, 8, 256), np.float32)
    r = qf.reshape(2, 2, 128, L).transpose(2, 0, 1, 3)  # [128, kc, pl, L]
    for kc in range(2):
        blk = r[:, kc].reshape(128, 2, 8, 128)  # [128, pl, lc, 128]
        sta[:, kc] = _interleave_stationary(blk)
    return mov, sta.astype(f8)


def _host_prep(inputs):
    fl = lambda x: np.asarray(x, np.float32)

    def wT(w):
        return np.ascontiguousarray(fl(w).T).astype(bf16)

    def wT_r(w):
        return np.ascontiguousarray(
            wT(w).reshape(4, 128, 512).transpose(1, 0, 2)
        )

    def bcol(b):
        return np.ascontiguousarray(fl(b).reshape(-1, 128).T.astype(np.float32))

    ipw = fl(inputs["in_proj_w"])
    ipb = fl(inputs["in_proj_b"])

    # fold V-projection biases into downstream projection biases
    b_oi = fl(inputs["b_out_img"]) + 0.5 * (
        fl(inputs["b_v_img"]) + fl(inputs["b_v_txt"])
    ) @ fl(inputs["w_out_img"]).T
    b_ot = fl(inputs["b_out_txt"]) + 0.5 * (
        fl(inputs["b_v_img"]) + fl(inputs["b_v_txt"])
    ) @ fl(inputs["w_out_txt"]).T
    b_op = fl(inputs["out_proj_b"]) + ipb[2 * H :] @ fl(inputs["out_proj_w"]).T

    w_cat = wT(inputs["w_cat"])  # [1024, 512]
    shared = {
        "w8i_qim": _prep_w8i(fl(inputs["w_q_img"])),
        "w8i_kim": _prep_w8i(fl(inputs["w_k_img"])),
        "w8i_qtx": _prep_w8i(fl(inputs["w_q_txt"])),
        "w8i_ktx": _prep_w8i(fl(inputs["w_k_txt"])),
        "w8i_ipq": _prep_w8i(ipw[0:H]),
        "w8i_ipk": _prep_w8i(ipw[H : 2 * H]),
        "w8v_im": _prep_w8v(fl(inputs["w_v_img"])),
        "w8v_tx": _prep_w8v(fl(inputs["w_v_txt"])),
        "w_oim": wT_r(inputs["w_out_img"]),
        "w_otx": wT_r(inputs["w_out_txt"]),
        "w_cat": np.ascontiguousarray(w_cat.reshape(8, 128, 512).transpose(1, 0, 2)),
        "w_ipv": wT_r(ipw[2 * H : 3 * H]),
        "w_op": wT_r(inputs["out_proj_w"]),
        "b_qim": bcol(inputs["b_q_img"]),
        "b_kim": bcol(inputs["b_k_img"]),
        "b_qtx": bcol(inputs["b_q_txt"]),
        "b_ktx": bcol(inputs["b_k_txt"]),
        "b_oim": bcol(b_oi),
        "b_otx": bcol(b_ot),
        "b_cat": bcol(inputs["b_cat"]),
        "b_ipqk": bcol(ipb[0 : 2 * H]),
        "r_op": fl(b_op).astype(bf16).reshape(1, -1),
    }
    hs = fl(inputs["hidden_states"])
    tx = fl(inputs["text"])
    in_maps = []
    for c in range(N_CORES):
        m = dict(shared)
        m["x8"], m["x8i"] = _prep_x8(hs[c])
        m["t8"], m["t8i"] = _prep_x8(tx[c])
        in_maps.append(m)
    return in_maps


def kernel(**inputs):
    nc = _build_program()
    in_maps = _host_prep(inputs)
    res = run_bass_kernel_spmd(nc, in_maps, core_ids=list(range(N_CORES)))
    out = np.stack([res.results[c]["out"] for c in range(N_CORES)])
    return out.astype(np.float32)


# revision 23
# speedup vs baseline: 1.2139x; 1.2139x over previous
"""Trainium2 Bass kernel for the dual-modality dense transformer block.

Problem (hardcoded shapes): B=8, L=1024, H=512, NH=8, HD=64.
  - 6 linear projections (q/k/v for img and txt streams)
  - 4 full attentions: (q_img,KV_img), (q_txt,KV_txt), (q_img,KV_txt), (q_txt,KV_img)
  - out_img/out_txt linears on the averaged contexts, concat + cat linear
  - attention pooling (nn.MultiheadAttention-style) + out_proj

Sharding: pure data-parallel over batch B=8 across the 8 NeuronCores.

Key device-level design (v2):
  - q/k/v and in_proj-q/k projections run in fp8e4 with
    perf_mode=DoubleRowSwInterleave (stationary host-interleaved, 2 k-planes
    per pass -> ~4x fewer PE cycles on those units). x/t inputs are shipped
    pre-quantized fp8 (x16) in both plane-major (moving) and
    interleaved-reversed (stationary) layouts. Weights fp8 (x256); evictions
    descale by 1/4096 and add the bias.
  - q/k tiles are stored fp8 (natural scale); QK matmuls are plain fp8
    (bf16-rate) with two heads packed per PE pass via tile_position row
    groups, which run concurrently (small-K row tiling).
  - exp(score) is split across two engines per (ih,p) block: ACT runs the
    exact table exp (scale=1/8 folds the score descale), DVE runs a custom
    cubic-polynomial (p(u/32)^4) single-instruction approximation. Each
    block's query rows use one implementation so the constant factor
    cancels in softmax.
  - softmax denominators come free from the PV matmul via a ones-column in
    the V tile (M=65); reciprocal on DVE (fast bit-trick op), broadcast to
    64 partitions by the (otherwise idle) GPSIMD partition_broadcast.
  - V-projection biases are folded into the *downstream* projection biases
    on the host (b_oi += 0.5*(b_v_img+b_v_txt) @ W_oi.T), so V tiles carry
    no bias matmuls.
  - bf16 everywhere else, fp32 PSUM. Measured accuracy vs the fp32
    reference: ~5e-3 of output absmax (budget 2e-2).
"""

import numpy as np
import ml_dtypes

import concourse.bass as bass
import concourse.tile as tile
from concourse import bacc, mybir
from concourse.bass_utils import run_bass_kernel_spmd
from concourse.dve_ops import RECIP_APPROX_FAST_CONSTS, RECIPROCAL_APPROX_FAST

B, L, H, NH, HD = 8, 1024, 512, 8, 64
BF = mybir.dt.bfloat16
F32 = mybir.dt.float32
F8 = mybir.dt.float8e4
Exp = mybir.ActivationFunctionType.Exp
Ident = mybir.ActivationFunctionType.Identity
bf16 = ml_dtypes.bfloat16
f8 = ml_dtypes.float8_e4m3
DRI = mybir.MatmulPerfMode.DoubleRowSwInterleave

N_CORES = 8

# ---------------- custom DVE exp op (registered at import) ----------------
from concourse.dve_spec import Spec, Src0, C0, C1, C2, One, lower as _dve_lower, _has_src1
from concourse.dve_ops import DveOp, OPS as _DVE_OPS, CUSTOM_DVE_SPECS as _DVE_SPECS
from concourse.dve_ops import _SUB_OPCODE_FOR_NAME, _CUSTOM_DVE_ROW_BASE
from concourse.dve_uop import DveOpSpec


def _make_exp_op():
    if "EXP4_POLY_ANT" in _SUB_OPCODE_FOR_NAME:
        return next(o for o in _DVE_OPS if o.name == "EXP4_POLY_ANT")
    u = Src0
    p = ((C2 * u + C1) * u + C0) * u + One
    body = (p * p) * (p * p)

    def ref(in0, in1, s0, s1, imm2):
        x = in0.astype(np.float32)
        q = ((imm2 * x + s1) * x + s0) * x + 1.0
        q2 = q * q
        return q2 * q2

    spec = Spec(body=body, reference=ref)
    name = "EXP4_POLY_ANT"
    opcode = _CUSTOM_DVE_ROW_BASE + len(_DVE_OPS)
    shas = {}
    for ver in ("v3",):
        uops = _dve_lower(spec, ver=ver)
        shas[ver] = DveOpSpec(
            name=name, opcode=opcode, uops=uops, rd1_en=_has_src1(spec)
        ).sha(ver)
    op = DveOp(name, spec, subdim=False, uops_sha=shas)
    _DVE_OPS.append(op)
    _DVE_SPECS[name] = spec
    _SUB_OPCODE_FOR_NAME[name] = opcode
    return op


EXP4 = _make_exp_op()

# cubic fit of exp(x) ~= p(x/4)^4 on |x|<=3.8 (density-weighted toward the
# observed score distribution); c0 normalized to 1 (One) - the residual
# constant factor cancels in softmax row-normalization.
_EC = np.array([0.99919218, 1.00539871, 0.52221469, 0.15490101])
_EC = _EC / _EC[0]
# psum scores arrive as u = 8*score (q,k at natural scale, no 1/sqrt(HD)
# fold); y = score/4 = u/32
_G = 1.0 / 32.0
EXP_S0, EXP_S1, EXP_IMM2 = float(_EC[1] * _G), float(_EC[2] * _G ** 2), float(_EC[3] * _G ** 3)
ACT_EXP_SCALE = 1.0 / 8.0


def _dve_exp_block(attn_idx, ih, p):
    """Which (ih, p) exp blocks run on DVE (the rest on ACT)."""
    return (2 * ih + p + attn_idx) % 4 == 3


def _emit(tc, d):
    nc = tc.nc
    import contextlib

    ctx = contextlib.ExitStack()
    with ctx:
        const = ctx.enter_context(tc.tile_pool(name="const", bufs=1))
        acts = ctx.enter_context(tc.tile_pool(name="acts", bufs=1))
        spool = ctx.enter_context(tc.tile_pool(name="spool", bufs=2))
        opool = ctx.enter_context(tc.tile_pool(name="opool", bufs=1))
        expool = ctx.enter_context(tc.tile_pool(name="expool", bufs=2))
        small = ctx.enter_context(tc.tile_pool(name="small", bufs=2))
        pmm = ctx.enter_context(tc.tile_pool(name="pmm", bufs=2, space="PSUM"))
        pctx = ctx.enter_context(tc.tile_pool(name="pctx", bufs=2, space="PSUM"))

        def load(name, shape, dt, pool=const, tag=None, split=None):
            t = pool.tile(shape, dt, tag=tag or name)
            if split is None:
                nc.sync.dma_start(out=t, in_=d[name])
            else:
                # split the transfer across DMA queues on dim 1
                for c in range(shape[1]):
                    nc.sync.dma_start(out=t[:, c], in_=d[name][:, c])
            return t

        # ---- loads in first-use order ----
        x8 = load("x8", [128, 2, 2, L], F8, pool=acts, split=True)
        w8i_qim = load("w8i_qim", [128, 2, 4, 256], F8)
        b_qim = load("b_qim", [128, 4], F32)
        w8i_kim = load("w8i_kim", [128, 2, 4, 256], F8)
        b_kim = load("b_kim", [128, 4], F32)
        x8i = load("x8i", [128, 2, 8, 256], F8, pool=acts, split=True)
        w8v_im = load("w8v_im", [128, 2, 2, 512], F8)
        t8 = load("t8", [128, 2, 2, L], F8, pool=acts, split=True)
        w8i_qtx = load("w8i_qtx", [128, 2, 4, 256], F8)
        b_qtx = load("b_qtx", [128, 4], F32)
        w8i_ktx = load("w8i_ktx", [128, 2, 4, 256], F8)
        b_ktx = load("b_ktx", [128, 4], F32)
        t8i = load("t8i", [128, 2, 8, 256], F8, pool=acts, split=True)
        w8v_tx = load("w8v_tx", [128, 2, 2, 512], F8)
        w_oim = load("w_oim", [128, 4, 512], BF, split=True)
        b_oim = load("b_oim", [128, 4], F32)
        w_otx = load("w_otx", [128, 4, 512], BF, split=True)
        b_otx = load("b_otx", [128, 4], F32)
        w_cat = load("w_cat", [128, 8, 512], BF, split=True)
        b_cat = load("b_cat", [128, 4], F32)
        w8i_ipq = load("w8i_ipq", [128, 2, 4, 256], F8)
        w8i_ipk = load("w8i_ipk", [128, 2, 4, 256], F8)
        b_ipqk = load("b_ipqk", [128, 8], F32)
        w_ipv = load("w_ipv", [128, 4, 512], BF, split=True)
        w_op = load("w_op", [128, 4, 512], BF, split=True)
        r_op = load("r_op", [1, 512], BF)

        ones_row = const.tile([1, 128], BF, tag="ones_row")
        nc.vector.memset(ones_row, 1.0)

        # ---- helpers ----
        def evict(eng, out, ps, scale, biascol):
            if eng == "act":
                nc.scalar.activation(out, ps, Ident, bias=biascol, scale=scale)
            else:
                if biascol is None:
                    if scale == 1.0:
                        nc.vector.tensor_copy(out=out, in_=ps)
                    else:
                        nc.vector.tensor_scalar_mul(out, ps, scale)
                else:
                    nc.vector.tensor_scalar(
                        out=out, in0=ps, scalar1=scale, scalar2=biascol,
                        op0=mybir.AluOpType.mult, op1=mybir.AluOpType.add,
                    )

        def proj_T8_m(dst, x8t, w8i, bias, bias_off, m, eng="act"):
            ps = pmm.tile([128, 1024], F32, tag="mm")
            for n in range(2):
                for kc in range(2):
                    nc.tensor.matmul(
                        ps[:, n * 512 : (n + 1) * 512],
                        w8i[:, kc, m, :].rearrange("p (m2 t) -> p m2 t", t=2),
                        x8t[:, kc, :, n * 512 : (n + 1) * 512],
                        start=(kc == 0),
                        stop=(kc == 1),
                        perf_mode=DRI,
                    )
            evict(eng, dst[:, m, :], ps, 1.0 / 4096.0,
                  bias[:, bias_off + m : bias_off + m + 1] if bias is not None else None)

        def proj_T8(dst, x8t, w8i, bias, bias_off, eng="act"):
            """fp8 DRI feature-major linear: dst[:, m, :] ~ fp8/bf16 [128,4,L]."""
            for m in range(4):
                proj_T8_m(dst, x8t, w8i, bias, bias_off, m, eng)

        def proj_N8_lc2(dst, x8it, w8v, lc2):
            ps = pmm.tile([128, 1024], F32, tag="mm")
            for h in range(2):
                lc = lc2 * 2 + h
                for kc in range(2):
                    nc.tensor.matmul(
                        ps[:, h * 512 : (h + 1) * 512],
                        x8it[:, kc, lc, :].rearrange("p (m2 t) -> p m2 t", t=2),
                        w8v[:, kc, :, :],
                        start=(kc == 0),
                        stop=(kc == 1),
                        perf_mode=DRI,
                        skip_group_check=True,
                    )
            nc.vector.tensor_scalar_mul(
                dst[:, lc2 * 2 : lc2 * 2 + 2, :, 0:64],
                ps.rearrange("p (a b) -> p a b", a=2),
                1.0 / 4096.0,
            )

        def proj_N8(dst, x8it, w8v):
            """fp8 DRI natural-orientation v-projection into ones-augmented
            layout dst [128, 8(jt), 8(lc-ish), 65]; no bias (host-folded)."""
            for lc2 in range(4):
                proj_N8_lc2(dst, x8it, w8v, lc2)

        def proj_T_m(dst, src, nk, w, bias, bias_off, m, eng="dve"):
            ps = pmm.tile([128, 1024], F32, tag="mm")
            for n in range(2):
                for k in range(nk):
                    nc.tensor.matmul(
                        ps[:, n * 512 : (n + 1) * 512],
                        w[:, k, m * 128 : (m + 1) * 128],
                        src[:, k, n * 512 : (n + 1) * 512],
                        start=(k == 0),
                        stop=(k == nk - 1),
                    )
            evict(eng, dst[:, m, :], ps, 1.0, bias[:, bias_off + m : bias_off + m + 1])

        def proj_T(dst, src, nk, w, bias, bias_off, eng="dve"):
            """bf16 feature-major linear (as baseline)."""
            for m in range(4):
                proj_T_m(dst, src, nk, w, bias, bias_off, m, eng)

        def proj_N(dst, src, w):
            """bf16 natural-orientation projection (pooling v), no bias."""
            for lc2 in range(4):
                ps = pmm.tile([128, 1024], F32, tag="mm")
                for h in range(2):
                    lc = lc2 * 2 + h
                    for k in range(4):
                        nc.tensor.matmul(
                            ps[:, h * 512 : (h + 1) * 512],
                            src[:, k, lc * 128 : (lc + 1) * 128],
                            w[:, k, :],
                            start=(k == 0),
                            stop=(k == 3),
                            skip_group_check=True,
                        )
                nc.vector.tensor_copy(
                    out=dst[:, lc2 * 2 : lc2 * 2 + 2, :, 0:64],
                    in_=ps.rearrange("p (a b) -> p a b", a=2),
                )

        # deferred-normalize queue (depth 2: two ctx psum tiles in flight)
        pending = []
        # jt tiles routed to the DVE poly-exp (rest: ACT table exp). Mixing
        # engines *within* a softmax row is numerically fine (the poly's
        # per-key weight noise averages out in PV; verified 4.8e-3 end to
        # end) and lets both engines drain the score psum ring concurrently.
        DVE_JTS = (2, 5)

        def flush_one():
            if pending:
                pending.pop(0)()

        def flush_all():
            while pending:
                pending.pop(0)()

        def emit_block(st, ih, p):
            """One (ih, p) block of an attention: QK + exp + PV + queue the
            normalize. st = (qT, kT, vN, s_dst, first, scale)."""
            qT, kT, vN, s_dst, first, scale = st
            i0 = ih * 512
            ex = expool.tile([128, 8, 1024], BF, tag="exp")
            for jt in range(8):
                ps = pmm.tile([128, 1024], F32, tag="mm")
                for hh in range(2):
                    nc.tensor.matmul(
                        ps[:, hh * 512 : (hh + 1) * 512],
                        kT[hh * 64 : (hh + 1) * 64, p, jt * 128 : (jt + 1) * 128],
                        qT[hh * 64 : (hh + 1) * 64, p, i0 : i0 + 512],
                        start=True,
                        stop=True,
                        tile_position=(hh * 64, 0),
                    )
                if jt in DVE_JTS:
                    nc.vector._custom_dve(
                        EXP4, out=ex[:, jt, :], in0=ps,
                        s0=EXP_S0, s1=EXP_S1, imm2=EXP_IMM2,
                    )
                else:
                    nc.scalar.activation(ex[:, jt, :], ps, Exp, scale=ACT_EXP_SCALE)
            if len(pending) >= 2:
                pending.pop(0)()
            cps = pctx.tile([128, 1024], F32, tag="ctx")
            for jt in range(8):
                for hh in range(2):
                    nc.tensor.matmul(
                        cps[0:65, hh * 512 : (hh + 1) * 512],
                        vN[:, jt, p * 2 + hh, :],
                        ex[:, jt, hh * 512 : (hh + 1) * 512],
                        start=(jt == 0),
                        stop=(jt == 7),
                    )

            def normalize(cps=cps, p=p, i0=i0, first=first, scale=scale, s_dst=s_dst):
                # scaled copy of the denominator rows to SBUF (the recip
                # bit-trick cannot read PSUM); scale=2 folds the reference's
                # 0.5 ctx averaging
                den = small.tile([1, 1024], F32, tag="den")
                nc.vector.tensor_scalar_mul(den, cps[64:65, :], scale)
                rc = small.tile([1, 1024], BF, tag="rc")
                cdve = RECIP_APPROX_FAST_CONSTS
                nc.vector._custom_dve(
                    RECIPROCAL_APPROX_FAST, out=rc, in0=den,
                    s0=cdve["s0"], s1=cdve["s1"], imm2=cdve["imm2"],
                )
                # partition-broadcast of the recips on GPSIMD (out tiles must
                # sit at partition base 0 - base-64 writes are broken)
                bcs0 = small.tile([64, 512], BF, tag="bcs0")
                bcs1 = small.tile([64, 512], BF, tag="bcs1")
                nc.gpsimd.partition_broadcast(bcs0, rc[0:1, 0:512])
                nc.gpsimd.partition_broadcast(bcs1, rc[0:1, 512:1024])
                o = s_dst[:, p, i0 : i0 + 512]
                if first:
                    nc.vector.tensor_mul(o[0:64, :], cps[0:64, 0:512], bcs0)
                    nc.vector.tensor_mul(o[64:128, :], cps[0:64, 512:1024], bcs1)
                else:
                    tmp = small.tile([128, 512], BF, tag="tmp")
                    nc.vector.tensor_mul(tmp[0:64, :], cps[0:64, 0:512], bcs0)
                    nc.vector.tensor_mul(tmp[64:128, :], cps[0:64, 512:1024], bcs1)
                    nc.vector.tensor_add(o, o, tmp)

            pending.append(normalize)

        def attention(st, fillers=None, start_slot=0):
            """Solo attention: 8 blocks with optional PE-filler closures
            emitted between blocks (leftovers drained at the end)."""
            fillers = list(fillers or [])
            for s in range(8):
                ih, p = s // 4, s % 4
                emit_block(st, ih, p)
                if fillers and s >= start_slot:
                    fillers.pop(0)()
            for f in fillers:
                f()

        def attention_pair(st_a, st_b, fillers=None):
            """Two independent attentions interleaved block-by-block; their
            exp streams keep both ACT and DVE busy while PE stays dense."""
            fillers = list(fillers or [])
            for ih in range(2):
                for p in range(4):
                    emit_block(st_a, ih, p)
                    emit_block(st_b, ih, p)
                    if fillers:
                        fillers.pop(0)()
            for f in fillers:
                f()

        # ---- the network ----
        q_im = acts.tile([128, 4, L], F8, tag="q_im")
        k_im = acts.tile([128, 4, L], F8, tag="k_im")
        v_im = acts.tile([128, 8, 8, 65], BF, tag="v_im")
        nc.vector.memset(v_im[:, :, :, 64:65], 1.0)
        q_tx = acts.tile([128, 4, L], F8, tag="q_tx")
        k_tx = acts.tile([128, 4, L], F8, tag="k_tx")
        v_tx = acts.tile([128, 8, 8, 65], BF, tag="v_tx")
        nc.vector.memset(v_tx[:, :, :, 64:65], 1.0)

        proj_T8(q_im, x8, w8i_qim, b_qim, 0, eng="act")
        proj_T8(k_im, x8, w8i_kim, b_kim, 0, eng="act")
        proj_N8(v_im, x8i, w8v_im)

        s_img = spool.tile([128, 4, L], BF, tag="s")
        s_txt = spool.tile([128, 4, L], BF, tag="s")

        # q_tx upfront so pair(A1, A4) can start; k_tx/v_tx are fillers
        proj_T8(q_tx, t8, w8i_qtx, b_qtx, 0, eng="act")

        def fT8(dst, x8t, w8i, bias, boff, ms, eng="act"):
            def f():
                for m in ms:
                    proj_T8_m(dst, x8t, w8i, bias, boff, m, eng)
            return f

        def fN8(dst, x8it, w8v, lc2s):
            def f():
                for lc2 in lc2s:
                    proj_N8_lc2(dst, x8it, w8v, lc2)
            return f

        p1_fillers = (
            [fT8(k_tx, t8, w8i_ktx, b_ktx, 0, [m]) for m in range(4)]
            + [fN8(v_tx, t8i, w8v_tx, [lc2]) for lc2 in range(4)]
        )
        st1 = (q_im, k_im, v_im, s_img, True, 2.0)   # ctx_img
        st4 = (q_tx, k_im, v_im, s_txt, True, 2.0)   # ctx_ti (first into s_txt)
        attention_pair(st1, st4, fillers=p1_fillers)

        st2 = (q_im, k_tx, v_tx, s_img, False, 2.0)  # ctx_it
        st3 = (q_tx, k_tx, v_tx, s_txt, False, 2.0)  # ctx_txt
        attention_pair(st2, st3)
        flush_all()

        cat_a = acts.tile([128, 4, L], BF, tag="cat_a")
        cat_b = acts.tile([128, 4, L], BF, tag="cat_b")
        proj_T(cat_a, s_img, 4, w_oim, b_oim, 0, eng="dve")
        proj_T(cat_b, s_txt, 4, w_otx, b_otx, 0, eng="act")

        # cat projection: dual eviction (bf16 out_t + fp8 out8 x16)
        out_t = opool.tile([128, 4, L], BF, tag="out")
        out8 = opool.tile([128, 2, 2, L], F8, tag="out8")
        for m in range(4):
            ps = pmm.tile([128, 1024], F32, tag="mm")
            for n in range(2):
                for k in range(8):
                    srck = cat_a if k < 4 else cat_b
                    nc.tensor.matmul(
                        ps[:, n * 512 : (n + 1) * 512],
                        w_cat[:, k, m * 128 : (m + 1) * 128],
                        srck[:, k % 4, n * 512 : (n + 1) * 512],
                        start=(k == 0),
                        stop=(k == 7),
                    )
            nc.vector.tensor_scalar_add(out_t[:, m, :], ps, b_cat[:, m : m + 1])
            # fp8 copy: (ps + b) * 16
            nc.vector.tensor_scalar(
                out=out8[:, m // 2, m % 2, :], in0=ps,
                scalar1=b_cat[:, m : m + 1], scalar2=16.0,
                op0=mybir.AluOpType.add, op1=mybir.AluOpType.mult,
            )

        q_pl = acts.tile([128, 4, L], F8, tag="q_im")
        k_pl = acts.tile([128, 4, L], F8, tag="q_tx")
        v_pl = acts.tile([128, 8, 8, 65], BF, tag="v_im")
        nc.vector.memset(v_pl[:, :, :, 64:65], 1.0)
        proj_T8(q_pl, out8, w8i_ipq, b_ipqk, 0, eng="act")
        proj_T8(k_pl, out8, w8i_ipk, b_ipqk, 4, eng="act")
        proj_N(v_pl, out_t, w_ipv)

        ctx_p = spool.tile([128, 4, L], BF, tag="s")

        def emit_out_proj(lcs):
            for lc in lcs:
                ps = pmm.tile([128, 1024], F32, tag="mm")
                for k in range(4):
                    nc.tensor.matmul(
                        ps[:, 0:512],
                        ctx_p[:, k, lc * 128 : (lc + 1) * 128],
                        w_op[:, k, :],
                        start=(k == 0),
                        stop=False,
                        skip_group_check=True,
                    )
                nc.tensor.matmul(
                    ps[:, 0:512], ones_row, r_op, start=False, stop=True,
                    skip_group_check=True,
                )
                res = small.tile([128, 512], F32, tag="res")
                nc.vector.tensor_copy(out=res, in_=ps[:, 0:512])
                nc.sync.dma_start(out=d["out"][lc * 128 : (lc + 1) * 128, :], in_=res)

        # pool attention: out_proj units become available per ih-half; with
        # the depth-2 normalize queue, ih0 is fully flushed after emitting
        # block (1,1) - attach lc 0-3 to the last slots, drain 4-7 after.
        st5 = (q_pl, k_pl, v_pl, ctx_p, True, 1.0)
        pool_fillers = [
            lambda: emit_out_proj([0, 1]),
            lambda: emit_out_proj([2, 3]),
        ]
        attention(st5, fillers=pool_fillers, start_slot=6)
        flush_all()
        emit_out_proj(range(4, 8))

        if "dbg_q_im" in d:
            for nm, t in (("dbg_q_im", q_im), ("dbg_k_im", k_im),
                          ("dbg_q_tx", q_tx), ("dbg_k_tx", k_tx)):
                nc.sync.dma_start(out=d[nm], in_=t)
            for nm, t in (("dbg_v_im", v_im), ("dbg_v_tx", v_tx)):
                nc.sync.dma_start(out=d[nm], in_=t)
            nc.sync.dma_start(out=d["dbg_s_img"], in_=s_img)
            nc.sync.dma_start(out=d["dbg_s_txt"], in_=s_txt)
            nc.sync.dma_start(out=d["dbg_out_t"], in_=out_t)
            nc.sync.dma_start(out=d["dbg_ctx_p"], in_=ctx_p)


_PROGRAM = None
DEBUG_DUMPS = False


def _build_program():
    global _PROGRAM
    if _PROGRAM is not None:
        return _PROGRAM
    nc = bacc.Bacc("TRN2", target_bir_lowering=False, debug=False)
    d = {}

    def din(name, shape, dt):
        d[name] = nc.dram_tensor(name, list(shape), dt, kind="ExternalInput").ap()

    din("x8", (128, 2, 2, L), F8)
    din("t8", (128, 2, 2, L), F8)
    din("x8i", (128, 2, 8, 256), F8)
    din("t8i", (128, 2, 8, 256), F8)
    for n in ("w8i_qim", "w8i_kim", "w8i_qtx", "w8i_ktx", "w8i_ipq", "w8i_ipk"):
        din(n, (128, 2, 4, 256), F8)
    for n in ("w8v_im", "w8v_tx"):
        din(n, (128, 2, 2, 512), F8)
    for n in ("w_oim", "w_otx", "w_ipv", "w_op"):
        din(n, (128, 4, 512), BF)
    din("w_cat", (128, 8, 512), BF)
    for n in ("b_qim", "b_kim", "b_qtx", "b_ktx", "b_oim", "b_otx", "b_cat"):
        din(n, (128, 4), F32)
    din("b_ipqk", (128, 8), F32)
    din("r_op", (1, 512), BF)
    d["out"] = nc.dram_tensor("out", [L, H], F32, kind="ExternalOutput").ap()
    if DEBUG_DUMPS:
        def dout(name, shape, dt):
            d[name] = nc.dram_tensor(name, list(shape), dt, kind="ExternalOutput").ap()
        for nm in ("dbg_q_im", "dbg_k_im", "dbg_q_tx", "dbg_k_tx"):
            dout(nm, (128, 4, L), F8)
        for nm in ("dbg_v_im", "dbg_v_tx"):
            dout(nm, (128, 8, 8, 65), BF)
        for nm in ("dbg_s_img", "dbg_s_txt", "dbg_out_t", "dbg_ctx_p"):
            dout(nm, (128, 4, L), BF)

    with tile.TileContext(nc) as tc:
        _emit(tc, d)
    nc.compile()
    _PROGRAM = nc
    return nc


def _interleave_stationary(a):
    """[128, pl2, nblk, 128] fp8-valued float -> [128, nblk, 256] interleaved
    (pair (plane0 col j, plane1 col j) adjacent, columns reversed)."""
    rev = a[:, :, :, ::-1]
    return rev.transpose(0, 2, 3, 1).reshape(a.shape[0], a.shape[2], 256)


def _prep_w8i(w):
    """w [H_out=512, H_in=512] -> DRI stationary [128, kc2, mb4, 256] fp8."""
    wt = np.ascontiguousarray(w.T) * 256.0  # [in, out]
    q = wt.astype(f8).astype(np.float32)
    r = q.reshape(2, 2, 128, 512).transpose(2, 0, 1, 3)  # [128, kc, pl, out]
    out = np.zeros((128, 2, 4, 256), np.float32)
    for kc in range(2):
        blk = r[:, kc].reshape(128, 2, 4, 128)  # [128, pl, mb, 128]
        out[:, kc] = _interleave_stationary(blk)
    return out.astype(f8)


def _prep_w8v(w):
    """w [H_out=512, H_in=512] -> DRI moving [128, kc2, pl2, 512] fp8."""
    wt = np.ascontiguousarray(w.T) * 256.0
    q = wt.astype(f8)
    return np.ascontiguousarray(
        q.reshape(2, 2, 128, 512).transpose(2, 0, 1, 3)
    )


def _prep_x8(x):
    """x [L, H] -> plane-major moving [128, kc2, pl2, L] fp8 and
    interleaved stationary [128, kc2, 8, 256] fp8 (both x16)."""
    xt = np.ascontiguousarray(x.T) * 16.0  # [H, L]
    q = xt.astype(f8)
    mov = np.ascontiguousarray(q.reshape(2, 2, 128, L).transpose(2, 0, 1, 3))
    qf = q.astype(np.float32)
    sta = np.zeros((128, 2, 8, 256), np.float32)
    r = qf.reshape(2, 2, 128, L).transpose(2, 0, 1, 3)  # [128, kc, pl, L]
    for kc in range(2):
        blk = r[:, kc].reshape(128, 2, 8, 128)  # [128, pl, lc, 128]
        sta[:, kc] = _interleave_stationary(blk)
    return mov, sta.astype(f8)


def _host_prep(inputs):
    fl = lambda x: np.asarray(x, np.float32)

    def wT(w):
        return np.ascontiguousarray(fl(w).T).astype(bf16)

    def wT_r(w):
        return np.ascontiguousarray(
            wT(w).reshape(4, 128, 512).transpose(1, 0, 2)
        )

    def bcol(b):
        return np.ascontiguousarray(fl(b).reshape(-1, 128).T.astype(np.float32))

    ipw = fl(inputs["in_proj_w"])
    ipb = fl(inputs["in_proj_b"])

    # fold V-projection biases into downstream projection biases
    b_oi = fl(inputs["b_out_img"]) + 0.5 * (
        fl(inputs["b_v_img"]) + fl(inputs["b_v_txt"])
    ) @ fl(inputs["w_out_img"]).T
    b_ot = fl(inputs["b_out_txt"]) + 0.5 * (
        fl(inputs["b_v_img"]) + fl(inputs["b_v_txt"])
    ) @ fl(inputs["w_out_txt"]).T
    b_op = fl(inputs["out_proj_b"]) + ipb[2 * H :] @ fl(inputs["out_proj_w"]).T

    w_cat = wT(inputs["w_cat"])  # [1024, 512]
    shared = {
        "w8i_qim": _prep_w8i(fl(inputs["w_q_img"])),
        "w8i_kim": _prep_w8i(fl(inputs["w_k_img"])),
        "w8i_qtx": _prep_w8i(fl(inputs["w_q_txt"])),
        "w8i_ktx": _prep_w8i(fl(inputs["w_k_txt"])),
        "w8i_ipq": _prep_w8i(ipw[0:H]),
        "w8i_ipk": _prep_w8i(ipw[H : 2 * H]),
        "w8v_im": _prep_w8v(fl(inputs["w_v_img"])),
        "w8v_tx": _prep_w8v(fl(inputs["w_v_txt"])),
        "w_oim": wT_r(inputs["w_out_img"]),
        "w_otx": wT_r(inputs["w_out_txt"]),
        "w_cat": np.ascontiguousarray(w_cat.reshape(8, 128, 512).transpose(1, 0, 2)),
        "w_ipv": wT_r(ipw[2 * H : 3 * H]),
        "w_op": wT_r(inputs["out_proj_w"]),
        "b_qim": bcol(inputs["b_q_img"]),
        "b_kim": bcol(inputs["b_k_img"]),
        "b_qtx": bcol(inputs["b_q_txt"]),
        "b_ktx": bcol(inputs["b_k_txt"]),
        "b_oim": bcol(b_oi),
        "b_otx": bcol(b_ot),
        "b_cat": bcol(inputs["b_cat"]),
        "b_ipqk": bcol(ipb[0 : 2 * H]),
        "r_op": fl(b_op).astype(bf16).reshape(1, -1),
    }
    hs = fl(inputs["hidden_states"])
    tx = fl(inputs["text"])
    in_maps = []
    for c in range(N_CORES):
        m = dict(shared)
        m["x8"], m["x8i"] = _prep_x8(hs[c])
        m["t8"], m["t8i"] = _prep_x8(tx[c])
        in_maps.append(m)
    return in_maps


def kernel(**inputs):
    nc = _build_program()
    in_maps = _host_prep(inputs)
    res = run_bass_kernel_spmd(nc, in_maps, core_ids=list(range(N_CORES)))
    out = np.stack([res.results[c]["out"] for c in range(N_CORES)])
    return out.astype(np.float32)


# revision 26
# speedup vs baseline: 1.2181x; 1.0034x over previous
"""Trainium2 Bass kernel for the dual-modality dense transformer block.

Problem (hardcoded shapes): B=8, L=1024, H=512, NH=8, HD=64.
  - 6 linear projections (q/k/v for img and txt streams)
  - 4 full attentions: (q_img,KV_img), (q_txt,KV_txt), (q_img,KV_txt), (q_txt,KV_img)
  - out_img/out_txt linears on the averaged contexts, concat + cat linear
  - attention pooling (nn.MultiheadAttention-style) + out_proj

Sharding: pure data-parallel over batch B=8 across the 8 NeuronCores.

Key device-level design (v2):
  - q/k/v and in_proj-q/k projections run in fp8e4 with
    perf_mode=DoubleRowSwInterleave (stationary host-interleaved, 2 k-planes
    per pass -> ~4x fewer PE cycles on those units). x/t inputs are shipped
    pre-quantized fp8 (x16) in both plane-major (moving) and
    interleaved-reversed (stationary) layouts. Weights fp8 (x256); evictions
    descale by 1/4096 and add the bias.
  - q/k tiles are stored fp8 (natural scale); QK matmuls are plain fp8
    (bf16-rate) with two heads packed per PE pass via tile_position row
    groups, which run concurrently (small-K row tiling).
  - exp(score) is split across two engines per (ih,p) block: ACT runs the
    exact table exp (scale=1/8 folds the score descale), DVE runs a custom
    cubic-polynomial (p(u/32)^4) single-instruction approximation. Each
    block's query rows use one implementation so the constant factor
    cancels in softmax.
  - softmax denominators come free from the PV matmul via a ones-column in
    the V tile (M=65); reciprocal on DVE (fast bit-trick op), broadcast to
    64 partitions by the (otherwise idle) GPSIMD partition_broadcast.
  - V-projection biases are folded into the *downstream* projection biases
    on the host (b_oi += 0.5*(b_v_img+b_v_txt) @ W_oi.T), so V tiles carry
    no bias matmuls.
  - bf16 everywhere else, fp32 PSUM. Measured accuracy vs the fp32
    reference: ~5e-3 of output absmax (budget 2e-2).
"""

import numpy as np
import ml_dtypes

import concourse.bass as bass
import concourse.tile as tile
from concourse import bacc, mybir
from concourse.bass_utils import run_bass_kernel_spmd
from concourse.dve_ops import RECIP_APPROX_FAST_CONSTS, RECIPROCAL_APPROX_FAST

B, L, H, NH, HD = 8, 1024, 512, 8, 64
BF = mybir.dt.bfloat16
F32 = mybir.dt.float32
F8 = mybir.dt.float8e4
Exp = mybir.ActivationFunctionType.Exp
Ident = mybir.ActivationFunctionType.Identity
bf16 = ml_dtypes.bfloat16
f8 = ml_dtypes.float8_e4m3
DRI = mybir.MatmulPerfMode.DoubleRowSwInterleave

N_CORES = 8

# ---------------- custom DVE exp op (registered at import) ----------------
from concourse.dve_spec import Spec, Src0, C0, C1, C2, One, lower as _dve_lower, _has_src1
from concourse.dve_ops import DveOp, OPS as _DVE_OPS, CUSTOM_DVE_SPECS as _DVE_SPECS
from concourse.dve_ops import _SUB_OPCODE_FOR_NAME, _CUSTOM_DVE_ROW_BASE
from concourse.dve_uop import DveOpSpec


def _make_exp_op():
    if "EXP4_POLY_ANT" in _SUB_OPCODE_FOR_NAME:
        return next(o for o in _DVE_OPS if o.name == "EXP4_POLY_ANT")
    u = Src0
    p = ((C2 * u + C1) * u + C0) * u + One
    body = (p * p) * (p * p)

    def ref(in0, in1, s0, s1, imm2):
        x = in0.astype(np.float32)
        q = ((imm2 * x + s1) * x + s0) * x + 1.0
        q2 = q * q
        return q2 * q2

    spec = Spec(body=body, reference=ref)
    name = "EXP4_POLY_ANT"
    opcode = _CUSTOM_DVE_ROW_BASE + len(_DVE_OPS)
    shas = {}
    for ver in ("v3",):
        uops = _dve_lower(spec, ver=ver)
        shas[ver] = DveOpSpec(
            name=name, opcode=opcode, uops=uops, rd1_en=_has_src1(spec)
        ).sha(ver)
    op = DveOp(name, spec, subdim=False, uops_sha=shas)
    _DVE_OPS.append(op)
    _DVE_SPECS[name] = spec
    _SUB_OPCODE_FOR_NAME[name] = opcode
    return op


EXP4 = _make_exp_op()

# cubic fit of exp(x) ~= p(x/4)^4 on |x|<=3.8 (density-weighted toward the
# observed score distribution); c0 normalized to 1 (One) - the residual
# constant factor cancels in softmax row-normalization.
_EC = np.array([0.99919218, 1.00539871, 0.52221469, 0.15490101])
_EC = _EC / _EC[0]
# psum scores arrive as u = 8*score (q,k at natural scale, no 1/sqrt(HD)
# fold); y = score/4 = u/32
_G = 1.0 / 32.0
EXP_S0, EXP_S1, EXP_IMM2 = float(_EC[1] * _G), float(_EC[2] * _G ** 2), float(_EC[3] * _G ** 3)
ACT_EXP_SCALE = 1.0 / 8.0


def _dve_exp_block(attn_idx, ih, p):
    """Which (ih, p) exp blocks run on DVE (the rest on ACT)."""
    return (2 * ih + p + attn_idx) % 4 == 3


def _emit(tc, d):
    nc = tc.nc
    import contextlib

    ctx = contextlib.ExitStack()
    with ctx:
        const = ctx.enter_context(tc.tile_pool(name="const", bufs=1))
        acts = ctx.enter_context(tc.tile_pool(name="acts", bufs=1))
        spool = ctx.enter_context(tc.tile_pool(name="spool", bufs=2))
        opool = ctx.enter_context(tc.tile_pool(name="opool", bufs=1))
        expool = ctx.enter_context(tc.tile_pool(name="expool", bufs=2))
        small = ctx.enter_context(tc.tile_pool(name="small", bufs=2))
        pmm = ctx.enter_context(tc.tile_pool(name="pmm", bufs=2, space="PSUM"))
        pctx = ctx.enter_context(tc.tile_pool(name="pctx", bufs=2, space="PSUM"))

        def load(name, shape, dt, pool=const, tag=None, split=None):
            t = pool.tile(shape, dt, tag=tag or name)
            if split is None:
                nc.sync.dma_start(out=t, in_=d[name])
            elif split == 2:
                # split on dims 1+2 (finer spread across DMA queues)
                for c in range(shape[1]):
                    for c2 in range(shape[2]):
                        nc.sync.dma_start(out=t[:, c, c2], in_=d[name][:, c, c2])
            else:
                # split the transfer across DMA queues on dim 1
                for c in range(shape[1]):
                    nc.sync.dma_start(out=t[:, c], in_=d[name][:, c])
            return t

        # ---- loads in first-use order ----
        x8 = load("x8", [128, 2, 2, L], F8, pool=acts, split=2)
        w8i_qim = load("w8i_qim", [128, 2, 4, 256], F8)
        b_qim = load("b_qim", [128, 4], F32)
        w8i_kim = load("w8i_kim", [128, 2, 4, 256], F8)
        b_kim = load("b_kim", [128, 4], F32)
        x8i = load("x8i", [128, 2, 8, 256], F8, pool=acts, split=True)
        w8v_im = load("w8v_im", [128, 2, 2, 512], F8)
        t8 = load("t8", [128, 2, 2, L], F8, pool=acts, split=True)
        w8i_qtx = load("w8i_qtx", [128, 2, 4, 256], F8)
        b_qtx = load("b_qtx", [128, 4], F32)
        w8i_ktx = load("w8i_ktx", [128, 2, 4, 256], F8)
        b_ktx = load("b_ktx", [128, 4], F32)
        t8i = load("t8i", [128, 2, 8, 256], F8, pool=acts, split=True)
        w8v_tx = load("w8v_tx", [128, 2, 2, 512], F8)
        w_oim = load("w_oim", [128, 4, 512], BF, split=True)
        b_oim = load("b_oim", [128, 4], F32)
        w_otx = load("w_otx", [128, 4, 512], BF, split=True)
        b_otx = load("b_otx", [128, 4], F32)
        w_cat = load("w_cat", [128, 8, 512], BF, split=True)
        b_cat = load("b_cat", [128, 4], F32)
        w8i_ipq = load("w8i_ipq", [128, 2, 4, 256], F8)
        w8i_ipk = load("w8i_ipk", [128, 2, 4, 256], F8)
        b_ipqk = load("b_ipqk", [128, 8], F32)
        w_ipv = load("w_ipv", [128, 4, 512], BF, split=True)
        w_op = load("w_op", [128, 4, 512], BF, split=True)
        r_op = load("r_op", [1, 512], BF)

        ones_row = const.tile([1, 128], BF, tag="ones_row")
        nc.vector.memset(ones_row, 1.0)

        # ---- helpers ----
        def evict(eng, out, ps, scale, biascol):
            if eng == "act":
                nc.scalar.activation(out, ps, Ident, bias=biascol, scale=scale)
            else:
                if biascol is None:
                    if scale == 1.0:
                        nc.vector.tensor_copy(out=out, in_=ps)
                    else:
                        nc.vector.tensor_scalar_mul(out, ps, scale)
                else:
                    nc.vector.tensor_scalar(
                        out=out, in0=ps, scalar1=scale, scalar2=biascol,
                        op0=mybir.AluOpType.mult, op1=mybir.AluOpType.add,
                    )

        def proj_T8_m(dst, x8t, w8i, bias, bias_off, m, eng="act"):
            ps = pmm.tile([128, 1024], F32, tag="mm")
            for n in range(2):
                for kc in range(2):
                    nc.tensor.matmul(
                        ps[:, n * 512 : (n + 1) * 512],
                        w8i[:, kc, m, :].rearrange("p (m2 t) -> p m2 t", t=2),
                        x8t[:, kc, :, n * 512 : (n + 1) * 512],
                        start=(kc == 0),
                        stop=(kc == 1),
                        perf_mode=DRI,
                    )
            evict(eng, dst[:, m, :], ps, 1.0 / 4096.0,
                  bias[:, bias_off + m : bias_off + m + 1] if bias is not None else None)

        def proj_T8(dst, x8t, w8i, bias, bias_off, eng="act"):
            """fp8 DRI feature-major linear: dst[:, m, :] ~ fp8/bf16 [128,4,L]."""
            for m in range(4):
                proj_T8_m(dst, x8t, w8i, bias, bias_off, m, eng)

        def proj_N8_lc2(dst, x8it, w8v, lc2):
            ps = pmm.tile([128, 1024], F32, tag="mm")
            for h in range(2):
                lc = lc2 * 2 + h
                for kc in range(2):
                    nc.tensor.matmul(
                        ps[:, h * 512 : (h + 1) * 512],
                        x8it[:, kc, lc, :].rearrange("p (m2 t) -> p m2 t", t=2),
                        w8v[:, kc, :, :],
                        start=(kc == 0),
                        stop=(kc == 1),
                        perf_mode=DRI,
                        skip_group_check=True,
                    )
            nc.vector.tensor_scalar_mul(
                dst[:, lc2 * 2 : lc2 * 2 + 2, :, 0:64],
                ps.rearrange("p (a b) -> p a b", a=2),
                1.0 / 4096.0,
            )

        def proj_N8(dst, x8it, w8v):
            """fp8 DRI natural-orientation v-projection into ones-augmented
            layout dst [128, 8(jt), 8(lc-ish), 65]; no bias (host-folded)."""
            for lc2 in range(4):
                proj_N8_lc2(dst, x8it, w8v, lc2)

        def proj_T_m(dst, src, nk, w, bias, bias_off, m, eng="dve"):
            ps = pmm.tile([128, 1024], F32, tag="mm")
            for n in range(2):
                for k in range(nk):
                    nc.tensor.matmul(
                        ps[:, n * 512 : (n + 1) * 512],
                        w[:, k, m * 128 : (m + 1) * 128],
                        src[:, k, n * 512 : (n + 1) * 512],
                        start=(k == 0),
                        stop=(k == nk - 1),
                    )
            evict(eng, dst[:, m, :], ps, 1.0, bias[:, bias_off + m : bias_off + m + 1])

        def proj_T(dst, src, nk, w, bias, bias_off, eng="dve"):
            """bf16 feature-major linear (as baseline)."""
            for m in range(4):
                proj_T_m(dst, src, nk, w, bias, bias_off, m, eng)

        def proj_N(dst, src, w):
            """bf16 natural-orientation projection (pooling v), no bias."""
            for lc2 in range(4):
                ps = pmm.tile([128, 1024], F32, tag="mm")
                for h in range(2):
                    lc = lc2 * 2 + h
                    for k in range(4):
                        nc.tensor.matmul(
                            ps[:, h * 512 : (h + 1) * 512],
                            src[:, k, lc * 128 : (lc + 1) * 128],
                            w[:, k, :],
                            start=(k == 0),
                            stop=(k == 3),
                            skip_group_check=True,
                        )
                nc.vector.tensor_copy(
                    out=dst[:, lc2 * 2 : lc2 * 2 + 2, :, 0:64],
                    in_=ps.rearrange("p (a b) -> p a b", a=2),
                )

        # deferred-normalize queue (depth 2: two ctx psum tiles in flight)
        pending = []
        # jt tiles routed to the DVE poly-exp (rest: ACT table exp). Mixing
        # engines *within* a softmax row is numerically fine (the poly's
        # per-key weight noise averages out in PV; verified 4.8e-3 end to
        # end) and lets both engines drain the score psum ring concurrently.
        DVE_JTS = (2, 5)

        def flush_one():
            if pending:
                pending.pop(0)()

        def flush_all():
            while pending:
                pending.pop(0)()

        def emit_block(st, ih, p):
            """One (ih, p) block of an attention: QK + exp + PV + queue the
            normalize. st = (qT, kT, vN, s_dst, first, scale)."""
            qT, kT, vN, s_dst, first, scale = st
            i0 = ih * 512
            ex = expool.tile([128, 8, 1024], BF, tag="exp")
            for jt in range(8):
                ps = pmm.tile([128, 1024], F32, tag="mm")
                for hh in range(2):
                    nc.tensor.matmul(
                        ps[:, hh * 512 : (hh + 1) * 512],
                        kT[hh * 64 : (hh + 1) * 64, p, jt * 128 : (jt + 1) * 128],
                        qT[hh * 64 : (hh + 1) * 64, p, i0 : i0 + 512],
                        start=True,
                        stop=True,
                        tile_position=(hh * 64, 0),
                    )
                if jt in DVE_JTS:
                    nc.vector._custom_dve(
                        EXP4, out=ex[:, jt, :], in0=ps,
                        s0=EXP_S0, s1=EXP_S1, imm2=EXP_IMM2,
                    )
                else:
                    nc.scalar.activation(ex[:, jt, :], ps, Exp, scale=ACT_EXP_SCALE)
            if len(pending) >= 2:
                pending.pop(0)()
            cps = pctx.tile([128, 1024], F32, tag="ctx")
            for jt in range(8):
                for hh in range(2):
                    nc.tensor.matmul(
                        cps[0:65, hh * 512 : (hh + 1) * 512],
                        vN[:, jt, p * 2 + hh, :],
                        ex[:, jt, hh * 512 : (hh + 1) * 512],
                        start=(jt == 0),
                        stop=(jt == 7),
                    )

            def normalize(cps=cps, p=p, i0=i0, first=first, scale=scale, s_dst=s_dst):
                # scaled copy of the denominator rows to SBUF (the recip
                # bit-trick cannot read PSUM); scale=2 folds the reference's
                # 0.5 ctx averaging
                den = small.tile([1, 1024], F32, tag="den")
                nc.vector.tensor_scalar_mul(den, cps[64:65, :], scale)
                rc = small.tile([1, 1024], BF, tag="rc")
                cdve = RECIP_APPROX_FAST_CONSTS
                nc.vector._custom_dve(
                    RECIPROCAL_APPROX_FAST, out=rc, in0=den,
                    s0=cdve["s0"], s1=cdve["s1"], imm2=cdve["imm2"],
                )
                # partition-broadcast of the recips on GPSIMD (out tiles must
                # sit at partition base 0 - base-64 writes are broken)
                bcs0 = small.tile([64, 512], BF, tag="bcs0")
                bcs1 = small.tile([64, 512], BF, tag="bcs1")
                nc.gpsimd.partition_broadcast(bcs0, rc[0:1, 0:512])
                nc.gpsimd.partition_broadcast(bcs1, rc[0:1, 512:1024])
                o = s_dst[:, p, i0 : i0 + 512]
                if first:
                    nc.vector.tensor_mul(o[0:64, :], cps[0:64, 0:512], bcs0)
                    nc.vector.tensor_mul(o[64:128, :], cps[0:64, 512:1024], bcs1)
                else:
                    tmp = small.tile([128, 512], BF, tag="tmp")
                    nc.vector.tensor_mul(tmp[0:64, :], cps[0:64, 0:512], bcs0)
                    nc.vector.tensor_mul(tmp[64:128, :], cps[0:64, 512:1024], bcs1)
                    nc.vector.tensor_add(o, o, tmp)

            pending.append(normalize)

        def attention(st, fillers=None, start_slot=0):
            """Solo attention: 8 blocks with optional PE-filler closures
            emitted between blocks (leftovers drained at the end)."""
            fillers = list(fillers or [])
            for s in range(8):
                ih, p = s // 4, s % 4
                emit_block(st, ih, p)
                if fillers and s >= start_slot:
                    fillers.pop(0)()
            for f in fillers:
                f()

        def attention_pair(st_a, st_b, fillers=None):
            """Two independent attentions interleaved block-by-block; their
            exp streams keep both ACT and DVE busy while PE stays dense."""
            fillers = list(fillers or [])
            for ih in range(2):
                for p in range(4):
                    emit_block(st_a, ih, p)
                    emit_block(st_b, ih, p)
                    if fillers:
                        fillers.pop(0)()
            for f in fillers:
                f()

        # ---- the network ----
        q_im = acts.tile([128, 4, L], F8, tag="q_im")
        k_im = acts.tile([128, 4, L], F8, tag="k_im")
        v_im = acts.tile([128, 8, 8, 65], BF, tag="v_im")
        nc.vector.memset(v_im[:, :, :, 64:65], 1.0)
        q_tx = acts.tile([128, 4, L], F8, tag="q_tx")
        k_tx = acts.tile([128, 4, L], F8, tag="k_tx")
        v_tx = acts.tile([128, 8, 8, 65], BF, tag="v_tx")
        nc.vector.memset(v_tx[:, :, :, 64:65], 1.0)

        proj_T8(q_im, x8, w8i_qim, b_qim, 0, eng="act")
        proj_T8(k_im, x8, w8i_kim, b_kim, 0, eng="act")
        proj_N8(v_im, x8i, w8v_im)

        s_img = spool.tile([128, 4, L], BF, tag="s")
        s_txt = spool.tile([128, 4, L], BF, tag="s")

        # q_tx upfront so pair(A1, A4) can start; k_tx/v_tx are fillers
        proj_T8(q_tx, t8, w8i_qtx, b_qtx, 0, eng="act")

        def fT8(dst, x8t, w8i, bias, boff, ms, eng="act"):
            def f():
                for m in ms:
                    proj_T8_m(dst, x8t, w8i, bias, boff, m, eng)
            return f

        def fN8(dst, x8it, w8v, lc2s):
            def f():
                for lc2 in lc2s:
                    proj_N8_lc2(dst, x8it, w8v, lc2)
            return f

        p1_fillers = (
            [fT8(k_tx, t8, w8i_ktx, b_ktx, 0, [m]) for m in range(4)]
            + [fN8(v_tx, t8i, w8v_tx, [lc2]) for lc2 in range(4)]
        )
        st1 = (q_im, k_im, v_im, s_img, True, 2.0)   # ctx_img
        st4 = (q_tx, k_im, v_im, s_txt, True, 2.0)   # ctx_ti (first into s_txt)
        attention_pair(st1, st4, fillers=p1_fillers)

        st2 = (q_im, k_tx, v_tx, s_img, False, 2.0)  # ctx_it
        st3 = (q_tx, k_tx, v_tx, s_txt, False, 2.0)  # ctx_txt
        attention_pair(st2, st3)
        flush_all()

        cat_a = acts.tile([128, 4, L], BF, tag="cat_a")
        cat_b = acts.tile([128, 4, L], BF, tag="cat_b")
        proj_T(cat_a, s_img, 4, w_oim, b_oim, 0, eng="dve")
        proj_T(cat_b, s_txt, 4, w_otx, b_otx, 0, eng="act")

        # cat projection: dual eviction (bf16 out_t + fp8 out8 x16)
        out_t = opool.tile([128, 4, L], BF, tag="out")
        out8 = opool.tile([128, 2, 2, L], F8, tag="out8")
        for m in range(4):
            ps = pmm.tile([128, 1024], F32, tag="mm")
            for n in range(2):
                for k in range(8):
                    srck = cat_a if k < 4 else cat_b
                    nc.tensor.matmul(
                        ps[:, n * 512 : (n + 1) * 512],
                        w_cat[:, k, m * 128 : (m + 1) * 128],
                        srck[:, k % 4, n * 512 : (n + 1) * 512],
                        start=(k == 0),
                        stop=(k == 7),
                    )
            nc.vector.tensor_scalar_add(out_t[:, m, :], ps, b_cat[:, m : m + 1])
            # fp8 copy: (ps + b) * 16
            nc.vector.tensor_scalar(
                out=out8[:, m // 2, m % 2, :], in0=ps,
                scalar1=b_cat[:, m : m + 1], scalar2=16.0,
                op0=mybir.AluOpType.add, op1=mybir.AluOpType.mult,
            )

        q_pl = acts.tile([128, 4, L], F8, tag="q_im")
        k_pl = acts.tile([128, 4, L], F8, tag="q_tx")
        v_pl = acts.tile([128, 8, 8, 65], BF, tag="v_im")
        nc.vector.memset(v_pl[:, :, :, 64:65], 1.0)
        proj_T8(q_pl, out8, w8i_ipq, b_ipqk, 0, eng="act")
        proj_T8(k_pl, out8, w8i_ipk, b_ipqk, 4, eng="act")
        proj_N(v_pl, out_t, w_ipv)

        ctx_p = spool.tile([128, 4, L], BF, tag="s")

        def emit_out_proj(lcs, eng="dve"):
            for lc in lcs:
                ps = pmm.tile([128, 1024], F32, tag="mm")
                for k in range(4):
                    nc.tensor.matmul(
                        ps[:, 0:512],
                        ctx_p[:, k, lc * 128 : (lc + 1) * 128],
                        w_op[:, k, :],
                        start=(k == 0),
                        stop=False,
                        skip_group_check=True,
                    )
                nc.tensor.matmul(
                    ps[:, 0:512], ones_row, r_op, start=False, stop=True,
                    skip_group_check=True,
                )
                res = small.tile([128, 512], F32, tag="res")
                if eng == "act":
                    nc.scalar.copy(res, ps[:, 0:512])
                else:
                    nc.vector.tensor_copy(out=res, in_=ps[:, 0:512])
                nc.sync.dma_start(out=d["out"][lc * 128 : (lc + 1) * 128, :], in_=res)

        # pool attention: out_proj units become available per ih-half; with
        # the depth-2 normalize queue, ih0 is fully flushed after emitting
        # block (1,1) - attach lc 0-3 to the last slots, interleave the rest
        # with the final normalize flushes.
        st5 = (q_pl, k_pl, v_pl, ctx_p, True, 1.0)
        pool_fillers = [
            lambda: emit_out_proj([0, 1], eng="act"),
            lambda: emit_out_proj([2, 3], eng="act"),
        ]
        attention(st5, fillers=pool_fillers, start_slot=6)
        flush_all()
        emit_out_proj([4, 5], eng="act")
        emit_out_proj([6, 7], eng="dve")

        if "dbg_q_im" in d:
            for nm, t in (("dbg_q_im", q_im), ("dbg_k_im", k_im),
                          ("dbg_q_tx", q_tx), ("dbg_k_tx", k_tx)):
                nc.sync.dma_start(out=d[nm], in_=t)
            for nm, t in (("dbg_v_im", v_im), ("dbg_v_tx", v_tx)):
                nc.sync.dma_start(out=d[nm], in_=t)
            nc.sync.dma_start(out=d["dbg_s_img"], in_=s_img)
            nc.sync.dma_start(out=d["dbg_s_txt"], in_=s_txt)
            nc.sync.dma_start(out=d["dbg_out_t"], in_=out_t)
            nc.sync.dma_start(out=d["dbg_ctx_p"], in_=ctx_p)


_PROGRAM = None
DEBUG_DUMPS = False


def _build_program():
    global _PROGRAM
    if _PROGRAM is not None:
        return _PROGRAM
    nc = bacc.Bacc("TRN2", target_bir_lowering=False, debug=False)
    d = {}

    def din(name, shape, dt):
        d[name] = nc.dram_tensor(name, list(shape), dt, kind="ExternalInput").ap()

    din("x8", (128, 2, 2, L), F8)
    din("t8", (128, 2, 2, L), F8)
    din("x8i", (128, 2, 8, 256), F8)
    din("t8i", (128, 2, 8, 256), F8)
    for n in ("w8i_qim", "w8i_kim", "w8i_qtx", "w8i_ktx", "w8i_ipq", "w8i_ipk"):
        din(n, (128, 2, 4, 256), F8)
    for n in ("w8v_im", "w8v_tx"):
        din(n, (128, 2, 2, 512), F8)
    for n in ("w_oim", "w_otx", "w_ipv", "w_op"):
        din(n, (128, 4, 512), BF)
    din("w_cat", (128, 8, 512), BF)
    for n in ("b_qim", "b_kim", "b_qtx", "b_ktx", "b_oim", "b_otx", "b_cat"):
        din(n, (128, 4), F32)
    din("b_ipqk", (128, 8), F32)
    din("r_op", (1, 512), BF)
    d["out"] = nc.dram_tensor("out", [L, H], F32, kind="ExternalOutput").ap()
    if DEBUG_DUMPS:
        def dout(name, shape, dt):
            d[name] = nc.dram_tensor(name, list(shape), dt, kind="ExternalOutput").ap()
        for nm in ("dbg_q_im", "dbg_k_im", "dbg_q_tx", "dbg_k_tx"):
            dout(nm, (128, 4, L), F8)
        for nm in ("dbg_v_im", "dbg_v_tx"):
            dout(nm, (128, 8, 8, 65), BF)
        for nm in ("dbg_s_img", "dbg_s_txt", "dbg_out_t", "dbg_ctx_p"):
            dout(nm, (128, 4, L), BF)

    with tile.TileContext(nc) as tc:
        _emit(tc, d)
    nc.compile()
    _PROGRAM = nc
    return nc


def _interleave_stationary(a):
    """[128, pl2, nblk, 128] fp8-valued float -> [128, nblk, 256] interleaved
    (pair (plane0 col j, plane1 col j) adjacent, columns reversed)."""
    rev = a[:, :, :, ::-1]
    return rev.transpose(0, 2, 3, 1).reshape(a.shape[0], a.shape[2], 256)


def _prep_w8i(w):
    """w [H_out=512, H_in=512] -> DRI stationary [128, kc2, mb4, 256] fp8."""
    wt = np.ascontiguousarray(w.T) * 256.0  # [in, out]
    q = wt.astype(f8).astype(np.float32)
    r = q.reshape(2, 2, 128, 512).transpose(2, 0, 1, 3)  # [128, kc, pl, out]
    out = np.zeros((128, 2, 4, 256), np.float32)
    for kc in range(2):
        blk = r[:, kc].reshape(128, 2, 4, 128)  # [128, pl, mb, 128]
        out[:, kc] = _interleave_stationary(blk)
    return out.astype(f8)


def _prep_w8v(w):
    """w [H_out=512, H_in=512] -> DRI moving [128, kc2, pl2, 512] fp8."""
    wt = np.ascontiguousarray(w.T) * 256.0
    q = wt.astype(f8)
    return np.ascontiguousarray(
        q.reshape(2, 2, 128, 512).transpose(2, 0, 1, 3)
    )


def _prep_x8(x):
    """x [L, H] -> plane-major moving [128, kc2, pl2, L] fp8 and
    interleaved stationary [128, kc2, 8, 256] fp8 (both x16)."""
    xt = np.ascontiguousarray(x.T) * 16.0  # [H, L]
    q = xt.astype(f8)
    mov = np.ascontiguousarray(q.reshape(2, 2, 128, L).transpose(2, 0, 1, 3))
    qf = q.astype(np.float32)
    sta = np.zeros((128, 2, 8, 256), np.float32)
    r = qf.reshape(2, 2, 128, L).transpose(2, 0, 1, 3)  # [128, kc, pl, L]
    for kc in range(2):
        blk = r[:, kc].reshape(128, 2, 8, 128)  # [128, pl, lc, 128]
        sta[:, kc] = _interleave_stationary(blk)
    return mov, sta.astype(f8)


def _host_prep(inputs):
    fl = lambda x: np.asarray(x, np.float32)

    def wT(w):
        return np.ascontiguousarray(fl(w).T).astype(bf16)

    def wT_r(w):
        return np.ascontiguousarray(
            wT(w).reshape(4, 128, 512).transpose(1, 0, 2)
        )

    def bcol(b):
        return np.ascontiguousarray(fl(b).reshape(-1, 128).T.astype(np.float32))

    ipw = fl(inputs["in_proj_w"])
    ipb = fl(inputs["in_proj_b"])

    # fold V-projection biases into downstream projection biases
    b_oi = fl(inputs["b_out_img"]) + 0.5 * (
        fl(inputs["b_v_img"]) + fl(inputs["b_v_txt"])
    ) @ fl(inputs["w_out_img"]).T
    b_ot = fl(inputs["b_out_txt"]) + 0.5 * (
        fl(inputs["b_v_img"]) + fl(inputs["b_v_txt"])
    ) @ fl(inputs["w_out_txt"]).T
    b_op = fl(inputs["out_proj_b"]) + ipb[2 * H :] @ fl(inputs["out_proj_w"]).T

    w_cat = wT(inputs["w_cat"])  # [1024, 512]
    shared = {
        "w8i_qim": _prep_w8i(fl(inputs["w_q_img"])),
        "w8i_kim": _prep_w8i(fl(inputs["w_k_img"])),
        "w8i_qtx": _prep_w8i(fl(inputs["w_q_txt"])),
        "w8i_ktx": _prep_w8i(fl(inputs["w_k_txt"])),
        "w8i_ipq": _prep_w8i(ipw[0:H]),
        "w8i_ipk": _prep_w8i(ipw[H : 2 * H]),
        "w8v_im": _prep_w8v(fl(inputs["w_v_img"])),
        "w8v_tx": _prep_w8v(fl(inputs["w_v_txt"])),
        "w_oim": wT_r(inputs["w_out_img"]),
        "w_otx": wT_r(inputs["w_out_txt"]),
        "w_cat": np.ascontiguousarray(w_cat.reshape(8, 128, 512).transpose(1, 0, 2)),
        "w_ipv": wT_r(ipw[2 * H : 3 * H]),
        "w_op": wT_r(inputs["out_proj_w"]),
        "b_qim": bcol(inputs["b_q_img"]),
        "b_kim": bcol(inputs["b_k_img"]),
        "b_qtx": bcol(inputs["b_q_txt"]),
        "b_ktx": bcol(inputs["b_k_txt"]),
        "b_oim": bcol(b_oi),
        "b_otx": bcol(b_ot),
        "b_cat": bcol(inputs["b_cat"]),
        "b_ipqk": bcol(ipb[0 : 2 * H]),
        "r_op": fl(b_op).astype(bf16).reshape(1, -1),
    }
    hs = fl(inputs["hidden_states"])
    tx = fl(inputs["text"])
    in_maps = []
    for c in range(N_CORES):
        m = dict(shared)
        m["x8"], m["x8i"] = _prep_x8(hs[c])
        m["t8"], m["t8i"] = _prep_x8(tx[c])
        in_maps.append(m)
    return in_maps


def kernel(**inputs):
    nc = _build_program()
    in_maps = _host_prep(inputs)
    res = run_bass_kernel_spmd(nc, in_maps, core_ids=list(range(N_CORES)))
    out = np.stack([res.results[c]["out"] for c in range(N_CORES)])
    return out.astype(np.float32)
